# revision 1
# baseline (speedup 1.0000x reference)
# Multi-scale deformable attention kernel for TRN2 (per-core: one batch element).
#
# Pipeline per core:
#   Phase 1: v = value @ Wv + bv, written to DRAM "M2" in row-pair entry layout:
#            entry e (256B = 64 f32) of head h: [v_h(pos e-PAD), v_h(pos e-PAD+W_l)]
#            -> a 512B gather at entry (y0,x0) returns all 4 bilinear corners.
#   Phase 2 per 128-query chunk:
#            off/att matmuls -> softmax -> sampling coords -> per-point
#            entry idx (int16, dma_gather wrapped layout) + 4 corner coefs
#            -> 8x dma_gather (one per head) -> coef-mul + reduce -> out chunk
#            -> @ Wout + bout -> DRAM.
import sys

sys.path.insert(0, "/opt/trn_rl_repo")
import numpy as np

import concourse.bacc as bacc
import concourse.bass as bass
import concourse.mybir as mybir
import concourse.tile as tile
import bass_rust
from concourse.alu_op_type import AluOpType
from concourse.masks import make_identity

F32 = mybir.dt.float32
I32 = mybir.dt.int32
I16 = mybir.dt.int16
AX = mybir.AxisListType
AF = mybir.ActivationFunctionType

SHAPES = ((100, 168), (50, 84), (25, 42))
NH, NL, NP = 8, 3, 4
P8 = 2 * NP              # 8 sampling points per (head, level)
C, D = 256, 32
W_ = [w for h, w in SHAPES]
H_ = [h for h, w in SHAPES]
LVL_START = [0, 16800, 21000]
LVL_END = [16800, 21000, 22050]
L = 22050
PAD = W_[0] + 2          # 170 top-guard entries
NENT = PAD + L + 180     # 22400 entries per head map
HS = NENT * 2 * D        # head stride in f32 within M2
LQ = 1700
LQP = 1792               # 14 chunks of 128
NCH = LQP // 128
SLOTS = NH * NL * P8     # 192 (h,l,p) combos per query
NIDX = 24 * 128          # gather indices per (chunk, head)
# second-half write segments: (pos_lo, pos_hi, entry_shift)
B_SEGS = [(LVL_START[li], LVL_END[li], PAD - W_[li]) for li in range(3)] + [
    (LVL_END[0] - W_[1], LVL_END[0], PAD - W_[1]),
    (LVL_END[1] - W_[2], LVL_END[1], PAD - W_[2]),
]


def build_program(num_cores=8, dbg=False, mode='full'):
    nc = bacc.Bacc("TRN2", target_bir_lowering=False, debug=False,
                   num_devices=num_cores, num_swdge_queues=4)
    value = nc.dram_tensor("value", [L, 2 * C], F32, kind="ExternalInput")
    query = nc.dram_tensor("query", [LQP, C], F32, kind="ExternalInput")
    refp = nc.dram_tensor("refp", [LQP, 4 * NL], F32, kind="ExternalInput")
    consts = nc.dram_tensor("consts", [6 * SLOTS], F32, kind="ExternalInput")
    Wv = nc.dram_tensor("Wv", [2 * C, C], F32, kind="ExternalInput")
    bvr = nc.dram_tensor("bvr", [1, C], F32, kind="ExternalInput")
    Woff = nc.dram_tensor("Woff", [C, SLOTS * 2], F32, kind="ExternalInput")
    boffr = nc.dram_tensor("boffr", [1, SLOTS * 2], F32, kind="ExternalInput")
    Watt = nc.dram_tensor("Watt", [C, 96], F32, kind="ExternalInput")
    battr = nc.dram_tensor("battr", [1, 96], F32, kind="ExternalInput")
    Wout = nc.dram_tensor("Wout", [C, C], F32, kind="ExternalInput")
    boutr = nc.dram_tensor("boutr", [1, C], F32, kind="ExternalInput")
    out = nc.dram_tensor("out", [LQP, C], F32, kind="ExternalOutput")
    m2 = nc.dram_tensor("m2", [NH * HS], F32, kind="Internal")
    if dbg:
        dbg_coef = nc.dram_tensor("dbg_coef", [LQP, SLOTS * 4], F32, kind="ExternalOutput")
        dbg_idx = nc.dram_tensor("dbg_idx", [NCH, 128, SLOTS * 8], I16, kind="ExternalOutput")
        dbg_O = nc.dram_tensor("dbg_O", [LQP, C], F32, kind="ExternalOutput")

    m2e = m2.ap().rearrange("(e c) -> e c", c=2 * D)  # [NH*NENT, 64]

    from contextlib import ExitStack
    with tile.TileContext(nc) as tc:
      with ExitStack() as ctx:
        # ---------------- constant / parameter loads ----------------
        wp = ctx.enter_context(tc.tile_pool(name="wp", bufs=1))
        ident = wp.tile([128, 128], F32)
        make_identity(nc, ident[:])
        wv_t = [wp.tile([128, C], F32, tag=f"wv{k}", name=f"wv{k}") for k in range(4)]
        for k in range(4):
            nc.sync.dma_start(wv_t[k][:], Wv[128 * k:128 * (k + 1), :])
        woff_t = [wp.tile([128, SLOTS * 2], F32, tag=f"woff{k}", name=f"woff{k}") for k in range(2)]
        watt_t = [wp.tile([128, 96], F32, tag=f"watt{k}", name=f"watt{k}") for k in range(2)]
        wout_t = [wp.tile([128, C], F32, tag=f"wout{k}", name=f"wout{k}") for k in range(2)]
        for k in range(2):
            nc.sync.dma_start(woff_t[k][:], Woff[128 * k:128 * (k + 1), :])
            nc.sync.dma_start(watt_t[k][:], Watt[128 * k:128 * (k + 1), :])
            nc.sync.dma_start(wout_t[k][:], Wout[128 * k:128 * (k + 1), :])
        bv_t = wp.tile([1, C], F32)
        boff_t = wp.tile([1, SLOTS * 2], F32)
        batt_t = wp.tile([1, 96], F32)
        bout_t = wp.tile([1, C], F32)
        nc.sync.dma_start(bv_t[:], bvr[:])
        nc.sync.dma_start(boff_t[:], boffr[:])
        nc.sync.dma_start(batt_t[:], battr[:])
        nc.sync.dma_start(bout_t[:], boutr[:])
        ones_t = wp.tile([1, 128], F32)
        nc.gpsimd.memset(ones_t[:], 1.0)
        cst_row = wp.tile([1, 6 * SLOTS], F32)
        nc.sync.dma_start(cst_row[:], consts.ap().unsqueeze(0))
        cst = wp.tile([128, 6 * SLOTS], F32)
        nc.gpsimd.partition_broadcast(cst[:], cst_row[:])
        # const views [128, SLOTS]: order matches host packing
        WT = cst[:, 0 * SLOTS:1 * SLOTS]
        W1 = cst[:, 1 * SLOTS:2 * SLOTS]
        W2 = cst[:, 2 * SLOTS:3 * SLOTS]
        H1 = cst[:, 3 * SLOTS:4 * SLOTS]
        H2 = cst[:, 4 * SLOTS:5 * SLOTS]
        BS = cst[:, 5 * SLOTS:6 * SLOTS]
        zt = wp.tile([128, 352], F32)
        nc.gpsimd.memset(zt[:], 0.0)

        # ---------------- M2 guard memsets (regions never written below) ---
        # first halves of entries [0,170) per head + entry 22220 fully.
        for h in range(NH):
            d1 = m2e[h * NENT + 0:h * NENT + PAD, 0:D]
            nc.sync.dma_start(d1, zt[0:85, 0:64])
            d2 = m2e[h * NENT + PAD + L + 0:h * NENT + PAD + L + 180, :]
            nc.sync.dma_start(d2, zt[0:90, 0:128])
            d4 = m2e[h * NENT + 0:h * NENT + 2, D:2 * D]
            nc.sync.dma_start(d4, zt[0:1, 0:2 * D])
        # tail second halves [PAD+L-W2l, PAD+L): zero (coef-0 only)
        d3 = _m2_ap(m2, PAD + L - W_[2], W_[2], D)
        nc.sync.dma_start(d3, zt[0:84, 0:128])

        # ---------------- Phase 1: value projection + M2 build -------------
        p1 = ctx.enter_context(tc.tile_pool(name="p1", bufs=3))
        psum = ctx.enter_context(tc.tile_pool(name="psum", bufs=1, space="PSUM"))
        NT = (L + 127) // 128
        for t in range(NT):
            p0 = t * 128
            rows = min(128, L - p0)
            vin = p1.tile([128, 2 * C], F32, tag="vin")
            nc.sync.dma_start(vin[0:rows, :], value[p0:p0 + rows, :])
            vT = p1.tile([128, 4 * rows], F32, tag="vT")
            for k in range(4):
                pt = psum.tile([128, 128], F32, tag="tp", bufs=4)
                nc.tensor.transpose(pt[:, 0:rows], vin[0:rows, 128 * k:128 * (k + 1)], ident[0:rows, 0:rows])
                nc.scalar.copy(vT[:, k * rows:(k + 1) * rows], pt[:, 0:rows])
            ps = psum.tile([128, SLOTS * 2], F32, tag="mm", bufs=3, name="ps")
            for k in range(4):
                nc.tensor.matmul(ps[0:rows, 0:C], vT[:, k * rows:(k + 1) * rows], wv_t[k][:],
                                 start=(k == 0), stop=False)
            nc.tensor.matmul(ps[0:rows, 0:C], ones_t[:, 0:rows], bv_t[:], start=False, stop=True)
            vtile = p1.tile([128, C], F32, tag="vtile")
            nc.scalar.copy(vtile[0:rows, :], ps[0:rows, 0:C])
            # M2 DMA-A: first halves, entries PAD+p
            srcA = vtile[0:rows, :].rearrange("p (h c) -> p h c", c=D)
            dstA = m2e[0:0, :]  # placeholder replaced below via custom AP
            dstA = _m2_ap(m2, PAD + p0, rows, 0)
            nc.sync.dma_start(dstA, srcA)
            # M2 DMA-B: second halves, entries PAD+p-W_l per level, plus
            # gap-fill segments (finite garbage for coef-0 reads at level seams)
            for (slo, shi, shift) in B_SEGS:
                lo = max(p0, slo)
                hi = min(p0 + rows, shi)
                if lo >= hi:
                    continue
                srcB = vtile[lo - p0:hi - p0, :].rearrange("p (h c) -> p h c", c=D)
                dstB = _m2_ap(m2, lo + shift, hi - lo, D)
                nc.sync.dma_start(dstB, srcB)

        # ---------------- Phase 2: per-chunk query pipeline ----------------
        if mode == 'p1':
            px = ctx.enter_context(tc.tile_pool(name="px", bufs=1))
            for ch in range(NCH):
                ot = px.tile([128, C], F32, tag="ot")
                nc.sync.dma_start(ot[:], m2.ap()[ch * 128 * C:(ch + 1) * 128 * C].rearrange("(p c) -> p c", c=C))
                nc.sync.dma_start(out[ch * 128:(ch + 1) * 128, :], ot[:])

        p2 = ctx.enter_context(tc.tile_pool(name="p2", bufs=2))
        pg = ctx.enter_context(tc.tile_pool(name="pg", bufs=3))
        pt_ = ctx.enter_context(tc.tile_pool(name="pt", bufs=2))

        for ch in range(NCH if mode != 'p1' else 0):
            q0 = ch * 128
            qin = p2.tile([128, C], F32, tag="qin")
            nc.sync.dma_start(qin[:], query[q0:q0 + 128, :])
            rp = p2.tile([128, 4 * NL], F32, tag="rp")
            nc.sync.dma_start(rp[:], refp[q0:q0 + 128, :])
            qT = p2.tile([128, 256], F32, tag="qT")
            for k in range(2):
                pt2 = psum.tile([128, 128], F32, tag="tp", bufs=4, name="pt2")
                nc.tensor.transpose(pt2[:], qin[:, 128 * k:128 * (k + 1)], ident[:])
                nc.scalar.copy(qT[:, 128 * k:128 * (k + 1)], pt2[:])
            # off = q @ Woff + boff   [128, 384]
            pso = psum.tile([128, SLOTS * 2], F32, tag="mm", bufs=3, name="pso")
            for k in range(2):
                nc.tensor.matmul(pso[:], qT[:, 128 * k:128 * (k + 1)], woff_t[k][:],
                                 start=(k == 0), stop=False)
            nc.tensor.matmul(pso[:], ones_t[:], boff_t[:], start=False, stop=True)
            off = p2.tile([128, SLOTS * 2], F32, tag="off")
            nc.scalar.copy(off[:], pso[:])
            # att = q @ Watt + batt -> softmax over 12-groups -> aw [128, 96]
            psa = psum.tile([128, SLOTS * 2], F32, tag="mm", bufs=3, name="psa")
            for k in range(2):
                nc.tensor.matmul(psa[:, 0:96], qT[:, 128 * k:128 * (k + 1)], watt_t[k][:],
                                 start=(k == 0), stop=False)
            nc.tensor.matmul(psa[:, 0:96], ones_t[:], batt_t[:], start=False, stop=True)
            att = p2.tile([128, 96], F32, tag="att")
            nc.scalar.copy(att[:], psa[:, 0:96])
            rmax = p2.tile([128, 8], F32, tag="rmax")
            nc.vector.tensor_reduce(rmax[:], att[:].rearrange("q (h l) -> q h l", l=12), AX.X, AluOpType.max)
            nc.vector.tensor_tensor(att[:].rearrange("q (h l) -> q h l", l=12),
                                    att[:].rearrange("q (h l) -> q h l", l=12),
                                    rmax[:].unsqueeze(2).broadcast_to((128, 8, 12)), AluOpType.subtract)
            nc.scalar.activation(att[:], att[:], AF.Exp)
            rsum = p2.tile([128, 8], F32, tag="rsum")
            nc.vector.tensor_reduce(rsum[:], att[:].rearrange("q (h l) -> q h l", l=12), AX.X, AluOpType.add)
            nc.vector.reciprocal(rsum[:], rsum[:])
            aw = p2.tile([128, 96], F32, tag="aw")
            nc.vector.tensor_tensor(aw[:].rearrange("q (h l) -> q h l", l=12),
                                    att[:].rearrange("q (h l) -> q h l", l=12),
                                    rsum[:].unsqueeze(2).broadcast_to((128, 8, 12)), AluOpType.mult)

            # ---- sampling coords: X,Y [128, 192] in slot order s=(h,l,p8)
            X = p2.tile([128, SLOTS], F32, tag="X")
            Y = p2.tile([128, SLOTS], F32, tag="Y")
            for du in range(2):
                for xy in range(2):
                    T = (X if xy == 0 else Y)
                    for li in range(NL):
                        dst = T[:].rearrange("q (hl p) -> q hl p", p=P8)[:, li::NL, du * NP:(du + 1) * NP]
                        src0 = off[:].rearrange("q (hl pc) -> q hl pc", pc=16)[:, li::NL, 2 * du + xy:2 * du + xy + 13:4]
                        src1 = rp[:, 4 * li + 2 * du + xy].unsqueeze(1).unsqueeze(2).broadcast_to((128, NH, NP))
                        nc.vector.scalar_tensor_tensor(dst, src0, -0.5, src1, AluOpType.add, AluOpType.add)
            TX = p2.tile([128, SLOTS], F32, tag="TX")
            TY = p2.tile([128, SLOTS], F32, tag="TY")
            X0 = p2.tile([128, SLOTS], F32, tag="X0")
            Y0 = p2.tile([128, SLOTS], F32, tag="Y0")
            MAGIC = 12582912.0  # 1.5 * 2^23: (x+M)-M = round-to-nearest(x)
            nc.vector.tensor_scalar(TX[:], X[:], MAGIC, MAGIC, AluOpType.add, AluOpType.subtract)
            nc.vector.tensor_scalar(TY[:], Y[:], MAGIC, MAGIC, AluOpType.add, AluOpType.subtract)
            nc.vector.tensor_tensor(X0[:], TX[:], X[:], AluOpType.is_gt)
            nc.vector.tensor_tensor(Y0[:], TY[:], Y[:], AluOpType.is_gt)
            nc.vector.tensor_tensor(X0[:], TX[:], X0[:], AluOpType.subtract)  # floor(x)
            nc.vector.tensor_tensor(Y0[:], TY[:], Y0[:], AluOpType.subtract)
            nc.vector.tensor_tensor(TX[:], X[:], X0[:], AluOpType.subtract)   # frac
            nc.vector.tensor_tensor(TY[:], Y[:], Y0[:], AluOpType.subtract)
            # masks: m{x,y}{0,1} = min-form; then per corner (maskprod >= 0)*aw
            UX = p2.tile([128, SLOTS], F32, tag="UX")
            UY = p2.tile([128, SLOTS], F32, tag="UY")
            nc.vector.tensor_tensor(UX[:], W1, X0[:], AluOpType.subtract)   # W-1-x0
            nc.vector.tensor_tensor(UY[:], H1, Y0[:], AluOpType.subtract)
            MX0 = p2.tile([128, SLOTS], F32, tag="MX0")
            MY0 = p2.tile([128, SLOTS], F32, tag="MY0")
            MX1 = p2.tile([128, SLOTS], F32, tag="MX1")
            MY1 = p2.tile([128, SLOTS], F32, tag="MY1")
            nc.vector.tensor_tensor(MX0[:], X0[:], UX[:], AluOpType.min)
            nc.vector.tensor_tensor(MY0[:], Y0[:], UY[:], AluOpType.min)
            # x1 valid: min(x0+1, (W-2)-x0) >= 0
            UX2 = p2.tile([128, SLOTS], F32, tag="UX2")
            UY2 = p2.tile([128, SLOTS], F32, tag="UY2")
            nc.vector.tensor_tensor(UX2[:], W2, X0[:], AluOpType.subtract)
            nc.vector.tensor_tensor(UY2[:], H2, Y0[:], AluOpType.subtract)
            nc.vector.scalar_tensor_tensor(MX1[:], X0[:], 1.0, UX2[:], AluOpType.add, AluOpType.min)
            nc.vector.scalar_tensor_tensor(MY1[:], Y0[:], 1.0, UY2[:], AluOpType.add, AluOpType.min)
            # aw broadcast to slots: aw col (h*12 + l*4 + pp); slot (h,l,du,pp)
            awsx = p2.tile([128, SLOTS], F32, tag="awsx")
            axv = awsx[:].rearrange("q (hl dp) -> q hl dp", dp=P8)
            avv = aw[:].rearrange("q (hl p) -> q hl p", p=NP)
            nc.vector.tensor_copy(axv[:, :, 0:NP], avv)
            nc.vector.tensor_copy(axv[:, :, NP:P8], avv)
            # corner coefs -> coefx [128, 768] (slot-major, corner minor: TL,BL,TR,BR)
            A = p2.tile([128, SLOTS], F32, tag="A")    # 1-tx
            B = p2.tile([128, SLOTS], F32, tag="B")    # 1-ty
            nc.vector.tensor_scalar(A[:], TX[:], -1.0, 1.0, AluOpType.mult, AluOpType.add)
            nc.vector.tensor_scalar(B[:], TY[:], -1.0, 1.0, AluOpType.mult, AluOpType.add)
            coefx = pt_.tile([128, SLOTS * 4], F32, tag="coefx")
            cxv = coefx[:].rearrange("q (s c) -> q s c", c=4)
            vv = p2.tile([128, SLOTS], F32, tag="vv")
            wgt = p2.tile([128, SLOTS], F32, tag="wgt")
            for (ci, mx, my, wa, wb) in ((0, MX0, MY0, A, B), (1, MX0, MY1, A, TY),
                                         (2, MX1, MY0, TX, B), (3, MX1, MY1, TX, TY)):
                nc.vector.tensor_tensor(vv[:], mx[:], my[:], AluOpType.min)
                nc.vector.scalar_tensor_tensor(vv[:], vv[:], 0.0, awsx[:], AluOpType.is_ge, AluOpType.mult)
                nc.vector.tensor_tensor(wgt[:], wa[:], wb[:], AluOpType.mult)
                nc.vector.tensor_tensor(cxv[:, :, ci], wgt[:], vv[:], AluOpType.mult)
            # entry idx = BS + y0c*WT + x0c  (clamped)
            X0C = p2.tile([128, SLOTS], F32, tag="X0C")
            Y0C = p2.tile([128, SLOTS], F32, tag="Y0C")
            nc.vector.tensor_scalar(X0C[:], X0[:], -1.0, None, AluOpType.max)
            nc.vector.tensor_tensor(X0C[:], X0C[:], W1, AluOpType.min)
            nc.vector.tensor_scalar(Y0C[:], Y0[:], -1.0, None, AluOpType.max)
            nc.vector.tensor_tensor(Y0C[:], Y0C[:], H1, AluOpType.min)
            IDXF = p2.tile([128, SLOTS], F32, tag="IDXF")
            nc.vector.tensor_tensor(IDXF[:], Y0C[:], WT, AluOpType.mult)
            nc.vector.tensor_tensor(IDXF[:], IDXF[:], X0C[:], AluOpType.add)
            nc.vector.tensor_tensor(IDXF[:], IDXF[:], BS, AluOpType.add)
            IDX32 = p2.tile([128, SLOTS], I32, tag="IDX32")
            nc.vector.tensor_copy(IDX32[:], IDXF[:])
            IDX16 = p2.tile([128, SLOTS], I16, tag="IDX16")
            nc.vector.tensor_copy(IDX16[:], IDX32[:])
            # fold to wrapped layout: wrp[p%16, S*8+j] = IDX16[16j + p%16, S],
            # replicated across all 128 partitions. Partition starts must be
            # 32-aligned, so odd-j groups come via a 16-rotate stream_shuffle.
            T16 = p2.tile([128, SLOTS], I16, tag="T16")
            nc.vector.stream_shuffle(T16[:], IDX16[:], [(i + 16) % 32 for i in range(32)])
            stage = pt_.tile([128, SLOTS * 8], I16, tag="stage")
            nc.vector.memset(stage[0:32, :], 0)
            sv = stage[:].rearrange("p (s j) -> p s j", j=8)
            for k in range(4):
                nc.vector.tensor_copy(sv[0:16, :, 2 * k], IDX16[32 * k:32 * k + 16, :])
                nc.vector.tensor_copy(sv[0:16, :, 2 * k + 1], T16[32 * k:32 * k + 16, :])
            nc.vector.tensor_copy(stage[32:64, :], stage[0:32, :])
            nc.vector.tensor_copy(stage[64:96, :], stage[0:32, :])
            nc.vector.tensor_copy(stage[96:128, :], stage[0:32, :])
            wrp = pt_.tile([128, SLOTS * 8], I16, tag="wrp")
            nc.vector.stream_shuffle(wrp[:], stage[:], [i % 16 for i in range(32)])
            if dbg:
                nc.sync.dma_start(dbg_coef[q0:q0 + 128, :], coefx[:])
                nc.sync.dma_start(dbg_idx[ch], wrp[:])

            # ---- per-head gather + combine -> O [128, 256]
            O = p2.tile([128, C], F32, tag="O")
            for h in range(NH):
                G = pg.tile([128, 24 * 128], F32, tag="G")
                m2ap = _m2_gather_ap(m2, h)
                if mode == 'nog':
                    nc.vector.memset(G[:], 0.01)
                else:
                    nc.gpsimd.dma_gather(
                        G[:].rearrange("q (s e) -> q s e", e=128), m2ap,
                        wrp[:, 192 * h:192 * (h + 1)], NIDX, NIDX, 128,
                        elem_step=2 * D, queue_num=h % 4, single_packet=False)
                TMP = pg.tile([128, 24 * 128], F32, tag="TMP")
                cb = coefx[:, 96 * h:96 * (h + 1)].unsqueeze(2).broadcast_to((128, 96, 32))
                nc.vector.tensor_tensor(TMP[:].rearrange("q (sc c) -> q sc c", c=32),
                                        G[:].rearrange("q (sc c) -> q sc c", c=32),
                                        cb, AluOpType.mult)
                nc.vector.tensor_reduce(O[:, D * h:D * (h + 1)],
                                        TMP[:].rearrange("q (sc c) -> q c sc", c=32),
                                        AX.X, AluOpType.add)
            if dbg:
                nc.sync.dma_start(dbg_O[q0:q0 + 128, :], O[:])
            # ---- out = O @ Wout + bout
            OT = p2.tile([128, 256], F32, tag="OT")
            for k in range(2):
                pt3 = psum.tile([128, 128], F32, tag="tp", bufs=4, name="pt3")
                nc.tensor.transpose(pt3[:], O[:, 128 * k:128 * (k + 1)], ident[:])
                nc.scalar.copy(OT[:, 128 * k:128 * (k + 1)], pt3[:])
            pso2 = psum.tile([128, SLOTS * 2], F32, tag="mm", bufs=3, name="pso2")
            for k in range(2):
                nc.tensor.matmul(pso2[:, 0:C], OT[:, 128 * k:128 * (k + 1)], wout_t[k][:],
                                 start=(k == 0), stop=False)
            nc.tensor.matmul(pso2[:, 0:C], ones_t[:], bout_t[:], start=False, stop=True)
            OO = p2.tile([128, C], F32, tag="OO")
            nc.scalar.copy(OO[:], pso2[:, 0:C])
            nc.sync.dma_start(out[q0:q0 + 128, :], OO[:])

    nc.finalize()
    return nc


def _m2_ap(m2, e0, n, coff):
    """DRAM AP: entries [e0, e0+n) x 8 heads x D f32 at half-offset coff."""
    ap = m2.ap()
    ap.ap = bass_rust.VecI64Pair([[2 * D, n], [HS, NH], [1, D]])
    ap.offset = e0 * 2 * D + coff
    return ap


def _m2_gather_ap(m2, h):
    """Overlapping rows view for dma_gather: row e = 128 f32 at entry e, stride 64."""
    ap = m2.ap()
    ap.ap = bass_rust.VecI64Pair([[2 * D, NENT - 2], [1, 4 * D]])
    ap.offset = h * HS
    return ap


# ---------------- host-side wrapper ----------------
def prep_core_inputs(inputs, b):
    q = np.zeros((LQP, C), np.float32)
    q[:LQ] = inputs["query"][b]
    rl = inputs["ref_l"][b].transpose(0, 2, 1, 3).reshape(LQ, NL, 2)
    rr = inputs["ref_r"][b].transpose(0, 2, 1, 3).reshape(LQ, NL, 2)
    norm = np.array([[w, h] for h, w in SHAPES], np.float32)
    rp = np.zeros((LQP, NL, 4), np.float32)
    rp[:LQ, :, 0:2] = rl * norm
    rp[:LQ, :, 2:4] = rr * norm
    slot_l = np.repeat(np.tile(np.arange(NL), NH), P8).astype(np.float32)  # [(h,l,p8)] -> l
    Wl = np.array(W_, np.float32)[slot_l.astype(np.int32)]
    Hl = np.array(H_, np.float32)[slot_l.astype(np.int32)]
    Bs = (PAD + np.array(LVL_START, np.float32)[slot_l.astype(np.int32)])
    consts = np.concatenate([Wl, Wl - 1, Wl - 2, Hl - 1, Hl - 2, Bs]).astype(np.float32)
    return {
        "value": np.ascontiguousarray(inputs["value"][b]),
        "query": q,
        "refp": rp.reshape(LQP, 4 * NL),
        "consts": consts,
        "Wv": inputs["Wv"], "bvr": inputs["bv"][None, :],
        "Woff": inputs["Woff"], "boffr": inputs["boff"][None, :],
        "Watt": inputs["Watt"], "battr": inputs["batt"][None, :],
        "Wout": inputs["Wout"], "boutr": inputs["bout"][None, :],
    }


LAST_EXEC_NS = None


def kernel(**inputs):
    global LAST_EXEC_NS
    import os
    from concourse.bass_utils import run_bass_kernel_spmd
    nc = build_program(num_cores=8)
    in_maps = [prep_core_inputs(inputs, b) for b in range(8)]
    trace = bool(int(os.environ.get("DKA_TRACE", "0")))
    res = run_bass_kernel_spmd(nc, in_maps, core_ids=list(range(8)), trace=trace)
    LAST_EXEC_NS = res.exec_time_ns
    return np.stack([res.results[b]["out"][:LQ] for b in range(8)], 0)

